# revision 1
# baseline (speedup 1.0000x reference)
"""Trainium2 Bass kernel for ComputeAlignmentError.

reference math:
    t[b,i,j,k] = dot(coords[b,i] - origin[b,j], E[b,j,k])   (per pred/true)
    out[b,i,j] = sqrt(sum_k (t_pred - t_true)^2 + 1e-8)

Quadratic-form formulation:
    u[i]   = [pred_coords[i] (3), true_coords[i] (3), 1]            (7)
    A[j]   = rows_k [E_pred[j,k] (3), -E_true[j,k] (3), -c[j,k]]    (3x7)
    err2[i,j] = u^T (A^T A) u = sum_{p<=q} m28[i,pq] * G28[j,pq]
    with G28 off-diagonal entries pre-scaled by 2 (symmetric fold), K=28.
    out[i,j]  = sqrt(err2 + 1e-8)

O(n^2) work = one K=28 fp32r matmul (PE) + sqrt (ACT) + DMA out.
Sharding: output rows i split across 8 cores; frame prep replicated.

Layout: frames are DMA'd contiguously, so partition p holds frames
j = 16p + c (c = 0..15).  The permutation is undone for free inside the
PSUM->SBUF copies after the PE transposes (strided dst), so GT / output
are in true j order.
"""

import numpy as np

B = 2            # batches
N = 2048         # n residues
NCORES = 8
RPC = N // NCORES          # rows per core per batch = 256
P = 128                    # partitions
NCH = N // P               # j-chunks per batch = 16
G64 = 2 * B * NCH          # (t, b, c) groups = 64
EPS_NORM = 1e-8
EPS_ERR = 1e-8

# pq28 symmetric packing: block p holds (p,p), (p,p+1) .. (p,6)
OFF = [0]
for _p in range(7):
    OFF.append(OFF[-1] + (7 - _p))     # OFF[p] = start of block p; OFF[7] = 28

_cache = {}


def _build():
    import concourse.bass as bass
    import concourse.bacc as bacc
    import concourse.tile as tile
    import concourse.mybir as mybir
    from concourse.masks import make_identity

    F32 = mybir.dt.float32
    F32R = mybir.dt.float32r
    MUL = mybir.AluOpType.mult
    ADD = mybir.AluOpType.add
    SUB = mybir.AluOpType.subtract

    nc = bacc.Bacc("TRN2", target_bir_lowering=False, debug=False,
                   num_devices=NCORES)

    pc_d = nc.dram_tensor("pc", [B, RPC, 3], F32, kind="ExternalInput")
    tc_d = nc.dram_tensor("tcrd", [B, RPC, 3], F32, kind="ExternalInput")
    pf_d = nc.dram_tensor("pf", [B, N, 3, 3], F32, kind="ExternalInput")
    tf_d = nc.dram_tensor("tf", [B, N, 3, 3], F32, kind="ExternalInput")
    out_d = nc.dram_tensor("out", [B, RPC, N], F32, kind="ExternalOutput")

    def v(tileap, offset_elems, dims):
        """AP view: keep partition dim of `tileap`, custom free dims."""
        return bass.AP(tensor=tileap.tensor,
                       offset=tileap.offset + offset_elems,
                       ap=[tileap.ap[0]] + dims)

    with tile.TileContext(nc) as tc:
        with (
            tc.tile_pool(name="consts", bufs=1) as consts,
            tc.tile_pool(name="prep", bufs=1) as prep,
            tc.tile_pool(name="gkp", bufs=2) as gkp,
            tc.tile_pool(name="itile", bufs=4) as itile,
            tc.tile_pool(name="gt", bufs=1) as gtp,
            tc.tile_pool(name="ps_t", bufs=2, space="PSUM") as ps_t,
            tc.tile_pool(name="ps_mm", bufs=4, space="PSUM") as ps_mm,
            tc.tile_pool(name="outp", bufs=4) as outp,
        ):
            ident = consts.tile([P, P], F32)
            make_identity(nc, ident[:])
            eps_t = consts.tile([P, 1], F32)
            nc.vector.memset(eps_t[:], EPS_ERR)

            # ---- frames: contiguous DMA; partition p <- j = 16p + c
            # F[jp, t, b, c, e]; e = d*3 + pt (pt fastest)
            F = prep.tile([P, 2, B, NCH, 9], F32)
            for t, dram in enumerate((pf_d, tf_d)):
                for b in range(B):
                    src = bass.AP(tensor=dram, offset=b * N * 9,
                                  ap=[[144, P], [1, 144]])
                    nc.sync.dma_start(out=F[:, t, b], in_=src)

            # ---- per-i-tile u & m28 (independent of frames; runs early)
            MT = []
            for b in range(B):
                for r in range(RPC // P):
                    U = itile.tile([P, 7], F32, name=f"u{b}{r}", tag="u")
                    off = (b * RPC + r * P) * 3
                    nc.sync.dma_start(
                        out=U[:, 0:3],
                        in_=bass.AP(tensor=pc_d, offset=off, ap=[[3, P], [1, 3]]))
                    nc.sync.dma_start(
                        out=U[:, 3:6],
                        in_=bass.AP(tensor=tc_d, offset=off, ap=[[3, P], [1, 3]]))
                    nc.vector.memset(U[:, 6:7], 1.0)
                    M28 = itile.tile([P, 28], F32, name=f"m{b}{r}", tag="m")
                    for p in range(7):
                        # diag: u_p^2 ; offdiag: 2*u_p*u_q (symmetric fold)
                        nc.gpsimd.tensor_scalar_mul(
                            M28[:, OFF[p]:OFF[p] + 1], U[:, p:p + 1],
                            U[:, p:p + 1])
                        if p < 6:
                            nc.gpsimd.tensor_scalar(
                                M28[:, OFF[p] + 1:OFF[p + 1]], U[:, p + 1:7],
                                U[:, p:p + 1], 2.0, MUL, MUL)
                    tp_m = ps_t.tile([28, P], F32, name=f"tpm{b}{r}", tag="tpm")
                    nc.tensor.transpose(tp_m[:], M28[:], ident[:])
                    MTt = itile.tile([28, P], F32, name=f"mt{b}{r}", tag="mt")
                    nc.vector.tensor_copy(out=MTt[:], in_=tp_m[:])
                    MT.append(MTt)

            fap = F[:]
            def fpt(pt, extra):
                return v(fap, pt, [[9, G64]] + extra)

            # ---- frame bases (vectorized over g = (t,b,c), 64 groups)
            W1 = prep.tile([P, G64, 3], F32)
            W2 = prep.tile([P, G64, 3], F32)
            nc.vector.tensor_tensor(out=W1[:], in0=fpt(0, [[3, 3]]),
                                    in1=fpt(1, [[3, 3]]), op=SUB)
            nc.vector.tensor_tensor(out=W2[:], in0=fpt(2, [[3, 3]]),
                                    in1=fpt(1, [[3, 3]]), op=SUB)

            def normalize_pair(XA, XB, dsts):
                """dsts: list of (dst_ap,) for XA, XB normalized."""
                SQa = prep.tile([P, G64, 3], F32, name=f"sqa{id(XA) % 97}", tag="sqa")
                SQb = prep.tile([P, G64, 3], F32, name=f"sqb{id(XB) % 97}", tag="sqb")
                SS = prep.tile([P, 2, G64], F32, name=f"ss{id(XA) % 97}", tag="ss")
                nc.scalar.square(SQa[:], XA[:])
                nc.scalar.square(SQb[:], XB[:])
                nc.vector.tensor_reduce(out=SS[:, 0], in_=SQa[:],
                                        axis=mybir.AxisListType.X, op=ADD)
                nc.vector.tensor_reduce(out=SS[:, 1], in_=SQb[:],
                                        axis=mybir.AxisListType.X, op=ADD)
                NRM = prep.tile([P, 2, G64], F32, name=f"nrm{id(XA) % 97}", tag="nrm")
                nc.scalar.sqrt(NRM[:], SS[:])
                RC = prep.tile([P, 2, G64], F32, name=f"rc{id(XA) % 97}", tag="rc")
                nc.vector.tensor_scalar_max(RC[:], NRM[:], EPS_NORM)
                RCP = prep.tile([P, 2, G64], F32, name=f"rcp{id(XA) % 97}", tag="rcp")
                nc.vector.reciprocal(RCP[:], RC[:])
                for idx, X in enumerate((XA, XB)):
                    rcp_b = v(RCP[:], idx * G64, [[1, G64], [0, 3]])
                    nc.vector.tensor_tensor(out=dsts[idx], in0=X[:], in1=rcp_b,
                                            op=MUL)

            W1N = prep.tile([P, G64, 3], F32)
            W2N = prep.tile([P, G64, 3], F32)
            normalize_pair(W1, W2, [W1N[:], W2N[:]])

            S = prep.tile([P, G64, 3], F32)
            D = prep.tile([P, G64, 3], F32)
            nc.vector.tensor_tensor(out=S[:], in0=W1N[:], in1=W2N[:], op=ADD)
            nc.vector.tensor_tensor(out=D[:], in0=W2N[:], in1=W1N[:], op=SUB)

            EE = prep.tile([P, G64, 3, 3], F32)   # (g, k, d)
            e1_dst = v(EE[:], 0, [[9, G64], [1, 3]])
            e2_dst = v(EE[:], 3, [[9, G64], [1, 3]])
            normalize_pair(S, D, [e1_dst, e2_dst])

            # e3 = e1 x e2 (split across gpsimd / DVE)
            TA = prep.tile([P, 3, G64], F32)
            TB = prep.tile([P, 3, G64], F32)
            for x in range(3):
                y, z = (x + 1) % 3, (x + 2) % 3
                nc.gpsimd.tensor_tensor(out=TA[:, x], in0=EE[:, :, 0, y],
                                        in1=EE[:, :, 1, z], op=MUL)
                nc.vector.tensor_tensor(out=TB[:, x], in0=EE[:, :, 0, z],
                                        in1=EE[:, :, 1, y], op=MUL)
            for x in range(3):
                nc.vector.tensor_tensor(out=EE[:, :, 2, x], in0=TA[:, x],
                                        in1=TB[:, x], op=SUB)

            # ---- origin projections; CT = (o_t.E_t) - (o_p.E_p) = -c
            OP = prep.tile([P, G64, 3, 3], F32)
            orig_b = fpt(1, [[0, 3], [3, 3]])
            nc.vector.tensor_tensor(out=OP[:], in0=EE[:], in1=orig_b, op=MUL)
            OC = prep.tile([P, G64, 3], F32)
            nc.vector.tensor_reduce(out=OC[:], in_=OP[:],
                                    axis=mybir.AxisListType.X, op=ADD)
            CT = prep.tile([P, B * NCH, 3], F32)
            nc.vector.tensor_tensor(out=CT[:], in0=OC[:, B * NCH:],
                                    in1=OC[:, :B * NCH], op=SUB)

            # ---- A[bc, k, f7] = [Ep | -Et | -c]
            A = prep.tile([P, B * NCH, 3, 7], F32)
            a_ap = A[:]
            nc.gpsimd.tensor_copy(
                out=v(a_ap, 0, [[21, B * NCH], [7, 3], [1, 3]]),
                in_=EE[:, :B * NCH])
            nc.vector.tensor_scalar_mul(
                v(a_ap, 3, [[21, B * NCH], [7, 3], [1, 3]]),
                EE[:, B * NCH:], -1.0)
            nc.gpsimd.tensor_copy(
                out=v(a_ap, 6, [[21, B * NCH], [7, 3]]), in_=CT[:])

            # ---- per batch: G28, transposes, un-permuting copies, matmuls
            GT = [gtp.tile([28, N], F32, name=f"gt{b}", tag=f"gt{b}")
                  for b in range(B)]
            for b in range(B):
                GK = gkp.tile([P, NCH, 28, 3], F32, name=f"gk{b}", tag="gk")
                gk_ap = GK[:]
                aoff = b * NCH * 21
                for p in range(7):
                    # diag: (p,p)
                    nc.gpsimd.tensor_tensor(
                        out=v(gk_ap, OFF[p] * 3, [[84, NCH], [1, 3]]),
                        in0=v(a_ap, aoff + p, [[21, NCH], [7, 3]]),
                        in1=v(a_ap, aoff + p, [[21, NCH], [7, 3]]), op=MUL)
                    nq = 6 - p
                    if nq:
                        # offdiag: A_p * A_q for q = p+1..6 (x2 folded into m28)
                        nc.vector.tensor_tensor(
                            out=v(gk_ap, (OFF[p] + 1) * 3,
                                  [[84, NCH], [1, 3], [3, nq]]),
                            in0=v(a_ap, aoff + p, [[21, NCH], [7, 3], [0, nq]]),
                            in1=v(a_ap, aoff + p + 1,
                                  [[21, NCH], [7, 3], [1, nq]]),
                            op=MUL)
                G28 = gkp.tile([P, NCH, 28], F32, name=f"g28_{b}", tag="g28")
                nc.vector.tensor_reduce(out=G28[:], in_=GK[:],
                                        axis=mybir.AxisListType.X, op=ADD)

                gt_ap = GT[b][:]
                for t_i in range(4):           # four PSUM tiles of 4 chunks
                    tp4 = ps_t.tile([28, 512], F32, name=f"tp4_{b}{t_i}",
                                    tag="tp4")
                    for k in range(4):
                        nc.tensor.transpose(
                            tp4[:, k * 128:(k + 1) * 128],
                            G28[:, 4 * t_i + k, :], ident[:])
                    # un-permute: GT col j = 16p + (4*t_i + k)
                    src = v(tp4[:], 0, [[128, 4], [1, P]])
                    dst = v(gt_ap, 4 * t_i, [[1, 4], [16, P]])
                    nc.vector.tensor_copy(out=dst, in_=src)

                # matmuls for this batch's two i-tiles
                for r in range(RPC // P):
                    MTt = MT[b * (RPC // P) + r]
                    for ch in range(4):
                        mm = ps_mm.tile([P, 512], F32, name=f"mm{b}{r}{ch}",
                                        tag="mm")
                        nc.tensor.matmul(
                            mm[:], MTt[:],
                            GT[b][:, ch * 512:(ch + 1) * 512],
                            start=True, stop=True)
                        OT = outp.tile([P, 512], F32, name=f"ot{b}{r}{ch}",
                                       tag="ot")
                        nc.scalar.activation(
                            out=OT[:], in_=mm[:],
                            func=mybir.ActivationFunctionType.Sqrt,
                            bias=eps_t[:], scale=1.0)
                        dst = bass.AP(
                            tensor=out_d,
                            offset=(b * RPC + r * P) * N + ch * 512,
                            ap=[[N, P], [1, 512]])
                        nc.sync.dma_start(out=dst, in_=OT[:])

    nc.compile()
    return nc


def _get_nc():
    if "nc" not in _cache:
        _cache["nc"] = _build()
    return _cache["nc"]


def _in_maps(pred_coords, true_coords, pred_frames, true_frames):
    pc = np.ascontiguousarray(pred_coords, dtype=np.float32)
    tcd = np.ascontiguousarray(true_coords, dtype=np.float32)
    pf = np.ascontiguousarray(pred_frames, dtype=np.float32)
    tf = np.ascontiguousarray(true_frames, dtype=np.float32)
    maps = []
    for c in range(NCORES):
        sl = slice(c * RPC, (c + 1) * RPC)
        maps.append({
            "pc": np.ascontiguousarray(pc[:, sl]),
            "tcrd": np.ascontiguousarray(tcd[:, sl]),
            "pf": pf,
            "tf": tf,
        })
    return maps


def _assemble(results):
    full = np.empty((B, N, N), dtype=np.float32)
    for c in range(NCORES):
        full[:, c * RPC:(c + 1) * RPC, :] = results[c]["out"]
    return full


def run_hw(trace=False, **inputs):
    from concourse.bass_utils import run_bass_kernel_spmd
    nc = _get_nc()
    res = run_bass_kernel_spmd(nc, _in_maps(**inputs), list(range(NCORES)),
                               trace=trace)
    return _assemble(res.results), res


def kernel(**inputs):
    out, _ = run_hw(trace=False, **inputs)
    return out



# revision 8
# speedup vs baseline: 1.1748x; 1.1748x over previous
"""Trainium2 Bass kernel for ComputeAlignmentError.

reference math:
    t[b,i,j,k] = dot(coords[b,i] - origin[b,j], E[b,j,k])   (per pred/true)
    out[b,i,j] = sqrt(sum_k (t_pred - t_true)^2 + 1e-8)

Quadratic-form formulation:
    u[i]   = [pred_coords[i] (3), true_coords[i] (3), 1]            (7)
    A[j]   = rows_k [E_pred[j,k] (3), -E_true[j,k] (3), -c[j,k]]    (3x7)
    err2[i,j] = u^T (A^T A) u = sum_{p<=q} m28[i,pq] * G28[j,pq]
    with G28 off-diagonal entries pre-scaled by 2 (symmetric fold), K=28.
    out[i,j]  = sqrt(err2 + 1e-8)

PE: fp32r (E8M11) runs at 1 cycle/row vs fp32's 4.  Full fp32 accuracy is
recovered with an exact hi/lo split: m = m_hi + m_lo (both E8M11-exact),
G likewise, and one K=84 fp32r matmul computes
    m_hi.G_hi + m_lo.G_hi + m_hi.G_lo  (drops only the ~2^-24 m_lo.G_lo)
via rows [m_hi; m_lo; m_hi] x [G_hi; G_hi; G_lo].

Sharding: output rows i split across 8 cores; frame prep replicated.

Layout: frames are DMA'd contiguously, so partition p holds frames
j = 16p + c (c = 0..15).  The permutation is undone for free inside the
PSUM->SBUF copies after the PE transposes (strided dst), so GT / output
are in true j order.
"""

import numpy as np

B = 2            # batches
N = 2048         # n residues
NCORES = 8
RPC = N // NCORES          # rows per core per batch = 256
P = 128                    # partitions
NCH = N // P               # j-chunks per batch = 16
G64 = 2 * B * NCH          # (t, b, c) groups = 64
K3 = 84                    # 3x28: [hi; lo; hi] x [hi; hi; lo]
EPS_NORM = 1e-8
EPS_ERR = 1e-8

# pq28 symmetric packing: block p holds (p,p), (p,p+1) .. (p,6)
OFF = [0]
for _p in range(7):
    OFF.append(OFF[-1] + (7 - _p))     # OFF[p] = start of block p; OFF[7] = 28

_cache = {}


def _build():
    import concourse.bass as bass
    import concourse.bacc as bacc
    import concourse.tile as tile
    import concourse.mybir as mybir
    from concourse.masks import make_identity

    F32 = mybir.dt.float32
    F32R = mybir.dt.float32r
    MUL = mybir.AluOpType.mult
    ADD = mybir.AluOpType.add
    SUB = mybir.AluOpType.subtract

    nc = bacc.Bacc("TRN2", target_bir_lowering=False, debug=False,
                   num_devices=NCORES)

    pc_d = nc.dram_tensor("pc", [B, RPC, 3], F32, kind="ExternalInput")
    tc_d = nc.dram_tensor("tcrd", [B, RPC, 3], F32, kind="ExternalInput")
    pf_d = nc.dram_tensor("pf", [B, N, 3, 3], F32, kind="ExternalInput")
    tf_d = nc.dram_tensor("tf", [B, N, 3, 3], F32, kind="ExternalInput")
    out_d = nc.dram_tensor("out", [B, RPC, N], F32, kind="ExternalOutput")

    def v(tileap, offset_elems, dims):
        """AP view: keep partition dim of `tileap`, custom free dims."""
        return bass.AP(tensor=tileap.tensor,
                       offset=tileap.offset + offset_elems,
                       ap=[tileap.ap[0]] + dims)

    with tile.TileContext(nc) as tc:
        with (
            tc.tile_pool(name="consts", bufs=1) as consts,
            tc.tile_pool(name="prep", bufs=1) as prep,
            tc.tile_pool(name="gkp", bufs=2) as gkp,
            tc.tile_pool(name="itile", bufs=4) as itile,
            tc.tile_pool(name="gt", bufs=1) as gtp,
            tc.tile_pool(name="ps_t", bufs=2, space="PSUM") as ps_t,
            tc.tile_pool(name="ps_mm", bufs=4, space="PSUM") as ps_mm,
            tc.tile_pool(name="outp", bufs=4) as outp,
        ):
            identf = consts.tile([P, P], F32)
            make_identity(nc, identf[:])
            ident = consts.tile([P, P], F32R)
            nc.gpsimd.tensor_copy(out=ident[:], in_=identf[:])
            eps_t = consts.tile([P, 1], F32)
            nc.vector.memset(eps_t[:], EPS_ERR)

            # ---- frames: contiguous DMA; partition p <- j = 16p + c
            # F[jp, t, b, c, e]; e = d*3 + pt (pt fastest)
            F = prep.tile([P, 2, B, NCH, 9], F32)
            for t, dram in enumerate((pf_d, tf_d)):
                for b in range(B):
                    src = bass.AP(tensor=dram, offset=b * N * 9,
                                  ap=[[144, P], [1, 144]])
                    nc.sync.dma_start(out=F[:, t, b], in_=src)

            # ---- per-i-tile u & m28 -> M84 [m_hi | m_lo | m_hi] (fp32r)
            MT = []
            for b in range(B):
                for r in range(RPC // P):
                    U = itile.tile([P, 7], F32, name=f"u{b}{r}", tag="u")
                    off = (b * RPC + r * P) * 3
                    nc.sync.dma_start(
                        out=U[:, 0:3],
                        in_=bass.AP(tensor=pc_d, offset=off, ap=[[3, P], [1, 3]]))
                    nc.sync.dma_start(
                        out=U[:, 3:6],
                        in_=bass.AP(tensor=tc_d, offset=off, ap=[[3, P], [1, 3]]))
                    nc.vector.memset(U[:, 6:7], 1.0)
                    M28 = itile.tile([P, 28], F32, name=f"m{b}{r}", tag="m")
                    for p in range(7):
                        # diag: u_p^2 ; offdiag: 2*u_p*u_q (symmetric fold)
                        nc.gpsimd.tensor_scalar_mul(
                            M28[:, OFF[p]:OFF[p] + 1], U[:, p:p + 1],
                            U[:, p:p + 1])
                        if p < 6:
                            nc.gpsimd.tensor_scalar(
                                M28[:, OFF[p] + 1:OFF[p + 1]], U[:, p + 1:7],
                                U[:, p:p + 1], 2.0, MUL, MUL)
                    M84 = itile.tile([P, K3], F32R, name=f"m84{b}{r}", tag="m84")
                    # hi = round_r(m); lo = m - hi (exact in E8M11); dup hi
                    nc.gpsimd.tensor_copy(out=M84[:, 0:28], in_=M28[:])
                    nc.vector.tensor_tensor(out=M84[:, 28:56], in0=M28[:],
                                            in1=M84[:, 0:28].bitcast(F32),
                                            op=SUB)
                    nc.gpsimd.tensor_copy(out=M84[:, 56:84], in_=M84[:, 0:28])
                    tp_m = ps_t.tile([K3, P], F32R, name=f"tpm{b}{r}", tag="tpm")
                    nc.tensor.transpose(tp_m[:], M84[:], ident[:])
                    MTt = itile.tile([K3, P], F32R, name=f"mt{b}{r}", tag="mt")
                    nc.vector.tensor_copy(out=MTt[:], in_=tp_m[:])
                    MT.append(MTt)

            fap = F[:]
            def fpt(pt, extra):
                return v(fap, pt, [[9, G64]] + extra)

            # ---- frame bases (vectorized over g = (t,b,c), 64 groups)
            W1 = prep.tile([P, G64, 3], F32)
            W2 = prep.tile([P, G64, 3], F32)
            nc.vector.tensor_tensor(out=W1[:], in0=fpt(0, [[3, 3]]),
                                    in1=fpt(1, [[3, 3]]), op=SUB)
            nc.vector.tensor_tensor(out=W2[:], in0=fpt(2, [[3, 3]]),
                                    in1=fpt(1, [[3, 3]]), op=SUB)

            def normalize_pair(XA, XB, dsts):
                """dsts: list of (dst_ap,) for XA, XB normalized."""
                SQa = prep.tile([P, G64, 3], F32, name=f"sqa{id(XA) % 97}", tag="sqa")
                SQb = prep.tile([P, G64, 3], F32, name=f"sqb{id(XB) % 97}", tag="sqb")
                SS = prep.tile([P, 2, G64], F32, name=f"ss{id(XA) % 97}", tag="ss")
                nc.scalar.square(SQa[:], XA[:])
                nc.scalar.square(SQb[:], XB[:])
                nc.vector.tensor_reduce(out=SS[:, 0], in_=SQa[:],
                                        axis=mybir.AxisListType.X, op=ADD)
                nc.vector.tensor_reduce(out=SS[:, 1], in_=SQb[:],
                                        axis=mybir.AxisListType.X, op=ADD)
                NRM = prep.tile([P, 2, G64], F32, name=f"nrm{id(XA) % 97}", tag="nrm")
                nc.scalar.sqrt(NRM[:], SS[:])
                RC = prep.tile([P, 2, G64], F32, name=f"rc{id(XA) % 97}", tag="rc")
                nc.vector.tensor_scalar_max(RC[:], NRM[:], EPS_NORM)
                RCP = prep.tile([P, 2, G64], F32, name=f"rcp{id(XA) % 97}", tag="rcp")
                nc.vector.reciprocal(RCP[:], RC[:])
                for idx, X in enumerate((XA, XB)):
                    rcp_b = v(RCP[:], idx * G64, [[1, G64], [0, 3]])
                    nc.vector.tensor_tensor(out=dsts[idx], in0=X[:], in1=rcp_b,
                                            op=MUL)

            W1N = prep.tile([P, G64, 3], F32)
            W2N = prep.tile([P, G64, 3], F32)
            normalize_pair(W1, W2, [W1N[:], W2N[:]])

            S = prep.tile([P, G64, 3], F32)
            D = prep.tile([P, G64, 3], F32)
            nc.vector.tensor_tensor(out=S[:], in0=W1N[:], in1=W2N[:], op=ADD)
            nc.vector.tensor_tensor(out=D[:], in0=W2N[:], in1=W1N[:], op=SUB)

            EE = prep.tile([P, G64, 3, 3], F32)   # (g, k, d)
            e1_dst = v(EE[:], 0, [[9, G64], [1, 3]])
            e2_dst = v(EE[:], 3, [[9, G64], [1, 3]])
            normalize_pair(S, D, [e1_dst, e2_dst])

            # e3 = e1 x e2 (split across gpsimd / DVE)
            TA = prep.tile([P, 3, G64], F32)
            TB = prep.tile([P, 3, G64], F32)
            for x in range(3):
                y, z = (x + 1) % 3, (x + 2) % 3
                nc.gpsimd.tensor_tensor(out=TA[:, x], in0=EE[:, :, 0, y],
                                        in1=EE[:, :, 1, z], op=MUL)
                nc.vector.tensor_tensor(out=TB[:, x], in0=EE[:, :, 0, z],
                                        in1=EE[:, :, 1, y], op=MUL)
            for x in range(3):
                nc.vector.tensor_tensor(out=EE[:, :, 2, x], in0=TA[:, x],
                                        in1=TB[:, x], op=SUB)

            # ---- origin projections; CT = (o_t.E_t) - (o_p.E_p) = -c
            OP = prep.tile([P, G64, 3, 3], F32)
            orig_b = fpt(1, [[0, 3], [3, 3]])
            nc.vector.tensor_tensor(out=OP[:], in0=EE[:], in1=orig_b, op=MUL)
            OC = prep.tile([P, G64, 3], F32)
            nc.vector.tensor_reduce(out=OC[:], in_=OP[:],
                                    axis=mybir.AxisListType.X, op=ADD)
            CT = prep.tile([P, B * NCH, 3], F32)
            nc.vector.tensor_tensor(out=CT[:], in0=OC[:, B * NCH:],
                                    in1=OC[:, :B * NCH], op=SUB)

            # ---- A[bc, k, f7] = [Ep | -Et | -c]
            A = prep.tile([P, B * NCH, 3, 7], F32)
            a_ap = A[:]
            nc.gpsimd.tensor_copy(
                out=v(a_ap, 0, [[21, B * NCH], [7, 3], [1, 3]]),
                in_=EE[:, :B * NCH])
            nc.vector.tensor_scalar_mul(
                v(a_ap, 3, [[21, B * NCH], [7, 3], [1, 3]]),
                EE[:, B * NCH:], -1.0)
            nc.gpsimd.tensor_copy(
                out=v(a_ap, 6, [[21, B * NCH], [7, 3]]), in_=CT[:])

            # ---- per batch: G28 -> G84 [hi|hi|lo], transposes, matmuls
            GT = [gtp.tile([K3, N], F32R, name=f"gt{b}", tag=f"gt{b}")
                  for b in range(B)]
            for b in range(B):
                GK = gkp.tile([P, NCH, 28, 3], F32, name=f"gk{b}", tag="gk")
                gk_ap = GK[:]
                aoff = b * NCH * 21
                for p in range(7):
                    # diag: (p,p)
                    nc.gpsimd.tensor_tensor(
                        out=v(gk_ap, OFF[p] * 3, [[84, NCH], [1, 3]]),
                        in0=v(a_ap, aoff + p, [[21, NCH], [7, 3]]),
                        in1=v(a_ap, aoff + p, [[21, NCH], [7, 3]]), op=MUL)
                    nq = 6 - p
                    if nq:
                        # offdiag: A_p * A_q for q = p+1..6 (x2 folded into m28)
                        nc.vector.tensor_tensor(
                            out=v(gk_ap, (OFF[p] + 1) * 3,
                                  [[84, NCH], [1, 3], [3, nq]]),
                            in0=v(a_ap, aoff + p, [[21, NCH], [7, 3], [0, nq]]),
                            in1=v(a_ap, aoff + p + 1,
                                  [[21, NCH], [7, 3], [1, nq]]),
                            op=MUL)
                G28 = gkp.tile([P, NCH, 28], F32, name=f"g28_{b}", tag="g28")
                nc.vector.tensor_reduce(out=G28[:], in_=GK[:],
                                        axis=mybir.AxisListType.X, op=ADD)
                # hi/lo split -> G84 [hi | hi | lo] (fp32r, rounding on write)
                G84 = gkp.tile([P, NCH, K3], F32, name=f"g84_{b}", tag="g84")
                g84 = G84[:]
                hi_r = v(g84, 0, [[K3, NCH], [1, 28]]).bitcast(F32R)
                hi2_r = v(g84, 28, [[K3, NCH], [1, 28]]).bitcast(F32R)
                lo_r = v(g84, 56, [[K3, NCH], [1, 28]]).bitcast(F32R)
                hi_f = v(g84, 0, [[K3, NCH], [1, 28]])
                nc.gpsimd.tensor_copy(out=hi_r, in_=G28[:])
                nc.vector.tensor_tensor(out=lo_r, in0=G28[:], in1=hi_f, op=SUB)
                nc.gpsimd.tensor_copy(out=hi2_r, in_=hi_r)

                gt_ap = GT[b][:]
                for t_i in range(4):           # four PSUM tiles of 4 chunks
                    tp4 = ps_t.tile([K3, 512], F32R, name=f"tp4_{b}{t_i}",
                                    tag="tp4")
                    for k in range(4):
                        nc.tensor.transpose(
                            tp4[:, k * 128:(k + 1) * 128],
                            v(g84, (4 * t_i + k) * K3,
                              [[1, K3]]).bitcast(F32R), ident[:])
                    # un-permute: GT col j = 16p + (4*t_i + k)
                    src = v(tp4[:], 0, [[128, 4], [1, P]])
                    dst = v(gt_ap, 4 * t_i, [[1, 4], [16, P]])
                    nc.vector.tensor_copy(out=dst, in_=src)

                # matmuls for this batch's two i-tiles
                for r in range(RPC // P):
                    MTt = MT[b * (RPC // P) + r]
                    for ch in range(4):
                        mm = ps_mm.tile([P, 512], F32, name=f"mm{b}{r}{ch}",
                                        tag="mm")
                        nc.tensor.matmul(
                            mm[:], MTt[:],
                            GT[b][:, ch * 512:(ch + 1) * 512],
                            start=True, stop=True)
                        OT = outp.tile([P, 512], F32, name=f"ot{b}{r}{ch}",
                                       tag="ot")
                        nc.scalar.activation(
                            out=OT[:], in_=mm[:],
                            func=mybir.ActivationFunctionType.Sqrt,
                            bias=eps_t[:], scale=1.0)
                        dst = bass.AP(
                            tensor=out_d,
                            offset=(b * RPC + r * P) * N + ch * 512,
                            ap=[[N, P], [1, 512]])
                        nc.sync.dma_start(out=dst, in_=OT[:])

    nc.compile()
    return nc


def _get_nc():
    if "nc" not in _cache:
        _cache["nc"] = _build()
    return _cache["nc"]


def _in_maps(pred_coords, true_coords, pred_frames, true_frames):
    pc = np.ascontiguousarray(pred_coords, dtype=np.float32)
    tcd = np.ascontiguousarray(true_coords, dtype=np.float32)
    pf = np.ascontiguousarray(pred_frames, dtype=np.float32)
    tf = np.ascontiguousarray(true_frames, dtype=np.float32)
    maps = []
    for c in range(NCORES):
        sl = slice(c * RPC, (c + 1) * RPC)
        maps.append({
            "pc": np.ascontiguousarray(pc[:, sl]),
            "tcrd": np.ascontiguousarray(tcd[:, sl]),
            "pf": pf,
            "tf": tf,
        })
    return maps


def _assemble(results):
    full = np.empty((B, N, N), dtype=np.float32)
    for c in range(NCORES):
        full[:, c * RPC:(c + 1) * RPC, :] = results[c]["out"]
    return full


def run_hw(trace=False, **inputs):
    from concourse.bass_utils import run_bass_kernel_spmd
    nc = _get_nc()
    res = run_bass_kernel_spmd(nc, _in_maps(**inputs), list(range(NCORES)),
                               trace=trace)
    return _assemble(res.results), res


def kernel(**inputs):
    out, _ = run_hw(trace=False, **inputs)
    return out
